# revision 21
# baseline (speedup 1.0000x reference)
"""DepthToSpace (cell=4, 4 split groups) Trainium2 Bass kernel.

Full input x: [8, 64, 256, 256] f32 -> output [8, 4, 1024, 1024] f32.
out[b, s, 4h+r, 4w+c] = x[b, 16s + 4r + c, h, w]

Sharding: data parallel over batch — core b handles x[b] (16.8 MB in/out).

Per-core plan (pure data movement). HW facts measured with probe kernels:
  * 16 SDMA engines; SBUF AXI ~435 GB/s is the shared wall (dual queues /
    dual direction don't add bandwidth; compute traffic doesn't interfere).
  * 4 KB load descs: ~397 GB/s; 16 KB store descs: ~425 GB/s. 8 KB loads
    and 32 KB stores measured SLOWER. Flat 3D access patterns measured
    faster in-kernel than the equivalent 4D/5D forms.
  * GpSimd SWDGE enqueue: ~0.8 us per 512-desc DMA (Sync HWDGE: 5-11 us).
    Splitting loads across multiple queues drops the load rate ~6%
    (cross-queue descriptor interleave breaks within-DMA port balance),
    so ALL DMAs ride GpSimd's single queue in strict FIFO.
  * Shuffle copy (1024 elems/partition, strided 16 B writes):
    DVE 1.07 us, ACT 2.05 us, GpSimd 3.57 us.
  * Run-to-run bimodality (~94 us vs ~110 us) tracks ambient 8-core HBM
    alignment, not kernel structure; fast runs show ZERO DMA idle.

Layout: partition p = 4*hb + s holds input rows h in [8*hb, 8*hb+8) of
all 16 channels of split group s (partitions s::4 span all 16 ports).
  load : per (p, ch) 4 KB runs; per (group, h-half) one 512-desc DMA
         (outer dim hb=32 -> all 16 engines, 2 hb each). hj=0 loads
         first (shuffle starts at half-load); the last two hj=1 loads
         are enqueued between the first two stores, which shortens the
         pure-read window while keeping FIFO priority for the loads
         that gate chunks 4-7.
  store: y[s] rows [32hb+4j, 32hb+4j+4) are one contiguous 16 KB run
         per partition; chunk j is a single 128-desc DMA (all engines),
         enqueued by GpSimd once the chunk's shuffle units retire.
The shuffle Y[r, w, c] = X[4r+c, h8=j, w] stays INTRA-partition on
DVE+ACT+GPS (5:2:1 unit split; iterate [p, c, w] so the source reads
contiguous 1 KB runs), overlapped under the DMA. NYB=4 Y buffers give
~3 store-times of recycle slack.
"""

import sys

sys.path.insert(0, "/opt/trn_rl_repo")

import numpy as np

import concourse.bass as bass
import concourse.mybir as mybir
from concourse.bass_utils import run_bass_kernel_spmd

B, C, H, W = 8, 64, 256, 256
S = 4
CELL = 4  # sqrt(C // S)
CPG = C // S  # channels per group = 16
P = 128  # SBUF partitions
HB = 32  # h-blocks per group (partition p = 4*hb + s)
N_CORES = 8

NYB = 4  # Y buffers (16 KB each)
NCHUNK = 8  # store chunks (one h8 row each)

# Shuffle work units per chunk: (r, whalf) -> 8 units, DVE:ACT = 5:3.
# (ACT pole 3*1.02us stays under DVE's 5*0.64us; freeing GpSimd of copies
# drops its const-pool preamble ahead of the first DMA enqueue.)
ALL_UNITS = [(r, wh) for r in range(CELL) for wh in range(2)]
DVE_UNITS = ALL_UNITS[:5]
ACT_UNITS = ALL_UNITS[5:]
GPS_UNITS = []
WH = W // 2


def build_program():
    nc = bass.Bass()
    x = nc.declare_dram_parameter("x", [C, H, W], mybir.dt.float32, isOutput=False)
    y = nc.declare_dram_parameter(
        "y", [S, H * CELL, W * CELL], mybir.dt.float32, isOutput=True
    )

    from contextlib import ExitStack

    with ExitStack() as ctx:
        # X[p][ch, hj, h4*w]: 16*2*1024 f32 = 128 KB per partition
        Xt = ctx.enter_context(
            nc.sbuf_tensor("X", [P, CPG, 2, 4 * W], mybir.dt.float32)
        )
        # Y[b][p][r, w, c]: 4*256*4 f32 = 16 KB
        Yt = [
            ctx.enter_context(
                nc.sbuf_tensor(f"Y{i}", [P, CELL, W, CELL], mybir.dt.float32)
            )
            for i in range(NYB)
        ]
        inl = [ctx.enter_context(nc.semaphore(f"inl{i}")) for i in range(2)]
        outs = [ctx.enter_context(nc.semaphore(f"outs{i}")) for i in range(NYB)]
        shuf_v = ctx.enter_context(nc.semaphore("shuf_v"))
        shuf_a = ctx.enter_context(nc.semaphore("shuf_a"))
        shuf_g = ctx.enter_context(nc.semaphore("shuf_g"))
        block = ctx.enter_context(nc.Block(no_gpsimd_drain=True))

        # x viewed as [s, hj, hb, ch, (h4 w)] — flat 3D per-DMA APs
        xv = x.rearrange(
            "(s ch) (hb hj h4) w -> s hj hb ch (h4 w)", s=S, ch=CPG, hb=HB, hj=2, h4=4
        )

        # y store AP: [j][hb, s, (r w c)] — one 16 KB run per partition
        yv = y.rearrange(
            "s (hb j r) (w c) -> j hb s (r w c)", hb=HB, j=NCHUNK, r=CELL, c=CELL
        )

        # X as [p, r, c, hj, h4, w] for the shuffle (ch = 4r + c)
        xr_fn = lambda: Xt[:].rearrange(
            "p (r c) hj (h4 w) -> p r c hj h4 w", r=CELL, c=CELL, h4=4
        )

        def copy_aps(j, r, wh):
            # chunk j covers h8 = j; h8 = 4*hj + h4.
            # Iterate [p, c, w]: src reads contiguous along w (1 KB runs),
            # dst pays the 16 B-strided writes.
            hj, h4 = divmod(j, 4)
            wlo, whi = wh * WH, (wh + 1) * WH
            src = xr_fn()[:, r, :, hj, h4, wlo:whi]  # [p, c, w]
            dst = Yt[j % NYB][:, r, wlo:whi]  # [p, w, c]
            dst = dst.transpose([0, 2, 1])  # [p, c, w]
            return src, dst

        n_dve = len(DVE_UNITS)
        n_act = len(ACT_UNITS)

        @block.gpsimd
        def _(gps):
            # All DMAs ride GpSimd's queue in strict FIFO: fast SWDGE
            # enqueue, and stores can never steal engine time from the
            # loads that gate the rest of the pipeline.
            for s in range(S):
                gps.dma_start(
                    out=Xt[s::S, :, 0], in_=xv[s, 0]
                ).then_inc(inl[0], 16)
            for s in (0, 1):
                gps.dma_start(
                    out=Xt[s::S, :, 1], in_=xv[s, 1]
                ).then_inc(inl[1], 16)
            for j in range(NCHUNK):
                gps.wait_ge(inl[j // 4], 64)
                if j >= NYB:
                    gps.wait_ge(outs[j % NYB], 16 * (j // NYB))
                for r, wh in GPS_UNITS:
                    src, dst = copy_aps(j, r, wh)
                    gps.tensor_copy(out=dst, in_=src)
                gps.wait_ge(shuf_v, n_dve * (j + 1))
                gps.wait_ge(shuf_a, n_act * (j + 1))
                gps.dma_start(out=yv[j], in_=Yt[j % NYB][:]).then_inc(
                    outs[j % NYB], 16
                )
                if j < 2:
                    # Interleave the last two hj=1 loads between early stores:
                    # shortens the pure-read window and keeps FIFO priority
                    # for the loads that gate chunks 4-7.
                    gps.dma_start(
                        out=Xt[2 + j :: S, :, 1], in_=xv[2 + j, 1]
                    ).then_inc(inl[1], 16)
            for b in range(NYB):
                gps.wait_ge(outs[b], 16 * (NCHUNK // NYB))

        @block.vector
        def _(vector):
            for j in range(NCHUNK):
                vector.wait_ge(inl[j // 4], 64)
                if j >= NYB:
                    vector.wait_ge(outs[j % NYB], 16 * (j // NYB))
                for r, wh in DVE_UNITS:
                    src, dst = copy_aps(j, r, wh)
                    vector.tensor_copy(out=dst, in_=src).then_inc(shuf_v, 1)

        @block.scalar
        def _(scalar):
            for j in range(NCHUNK):
                scalar.wait_ge(inl[j // 4], 64)
                if j >= NYB:
                    scalar.wait_ge(outs[j % NYB], 16 * (j // NYB))
                for r, wh in ACT_UNITS:
                    src, dst = copy_aps(j, r, wh)
                    scalar.copy(out=dst, in_=src).then_inc(shuf_a, 1)

    return nc


def run_sharded(x: np.ndarray, trace: bool = False, warmup: bool = False):
    """Shard x over batch across 8 cores, run, gather. Returns (out, results)."""
    assert x.shape == (B, C, H, W), x.shape
    nc = build_program()
    in_maps = [{"x": np.ascontiguousarray(x[b])} for b in range(N_CORES)]
    res = run_bass_kernel_spmd(nc, in_maps, list(range(N_CORES)), trace=trace)
    out = np.stack([res.results[b]["y"] for b in range(N_CORES)], axis=0)
    return out.astype(x.dtype, copy=False), res


def kernel(**inputs: np.ndarray) -> np.ndarray:
    x = np.asarray(inputs["x"], dtype=np.float32)
    out, _ = run_sharded(x, trace=False)
    return out
